# revision 30
# baseline (speedup 1.0000x reference)
"""Causal self-attention (B=4, T=1024, D=2048, H=16) on 8 trn2 NeuronCores.

Sharding: data-parallel over batch (4) x tensor-parallel over heads (2).
Core c handles batch b = c//2, head-half hh = c%2 (heads hh*8 .. hh*8+8).

All gemms bf16 (fp8 fails the 2e-2 gate: logit/value quantization noise
concentrates in peaked-softmax rows). Perf comes from keeping TensorE
dense at the warm 2.4 GHz clock:
  - ~72 warm-up matmuls on a memset scratch run during the ~14us DMA
    spin-up so the HAM clock gate reaches 8/8 before real work starts.
  - x and wv stream as 512KB tiles interleaved across the sync and
    gpsimd DMA queues in v-gemm consumption order; scalar/vector queues
    issue no DMA, so PSUM-drain copies are never stuck behind descriptor
    waits.
  - per head the emission order is qk(h), scores(h), apply(h-1): the ACT
    exp stream of head h hides under apply(h-1) + qk(h+1). pT pools are
    double-buffered to allow the overlap.
  - softmax row sums use DVE pair-summed pT tiles, halving the ones-gemm
    stream (2560 vs 4608 cycles/head).

Per-core plan (fp32 PSUM accumulation everywhere):
  v      [t, c]  : lhsT = x k-tile slice [k,t], rhs = wv [k,c]
  per head h:
    qT/kT [d, t] : lhsT = w_{q,k} k-tile, rhs = x k-tile [128,512]
    sT    [tk,tq]: lhsT = kT block, rhs = qT slice (causal: tq >= 128*j)
    pT    = exp(scale * sT) via ACT (no max-subtraction; |s*scale| <= 7)
    diag blocks masked multiplicatively with an upper-triangular 0/1 mask
    sm    [tk,tq]: DVE pair-sums pT(2m)+pT(2m+1) for the row-sum gemm
    yT    [d, tq] += v_j-gemm: lhsT = v block, rhs = pT block (PSUM accum)
    r     [1, tq] += ones^T @ sm (softmax row sums, halved stream)
    1/r on [1,512] (DVE recip + NR) -> GpSimd bcast -> yT_norm = yT * rec
    pairwise AllGather of this head's yT (overlaps later heads' compute)
  out    [t, c_half]: accumulate 16 head-chunks against wp rows; the yT
  tiles are prefetched from the AllGather outputs in completion order
  (h-major), so only head 7's chunks can ever wait on the collective.

Host side: slice/transpose/cast/pack inputs per core, concat outputs.
"""

import numpy as np

import concourse.bass as bass
import concourse.mybir as mybir
import concourse.tile as tile
from concourse import bacc
from concourse.bass_utils import run_bass_kernel_spmd

B, T, D = 4, 1024, 2048
H, DH = 16, 128
N_CORES = 8
TP = 2                      # head-halves per batch
HPC = H // TP               # heads per core = 8
CPC = HPC * DH              # channels per core = 1024
KC = D // 128               # contraction chunks = 16
KP = KC // 2                # contraction pair chunks = 8
SCALE = 1.0 / float(np.sqrt(DH))
N_WARM = 24                 # PE warm-up matmuls during DMA spin-up

F32 = mybir.dt.float32
BF16 = mybir.dt.bfloat16

PAIRS = [[2 * i, 2 * i + 1] for i in range(B)]


def build_kernel():
    nc = bacc.Bacc("TRN2", target_bir_lowering=False, debug=False,
                   num_devices=N_CORES)

    # All inputs host-packed to device layout.
    # x: [p, m, i, t] = x.T[128*(2m+i)+p, t]
    x_ap = nc.dram_tensor("xT", [128, KP, 2, T], BF16,
                          kind="ExternalInput").ap()
    # wq/wk per head: [h, p, m, i, j] = w[128*(2m+i)+p, 128h+j]
    wq_ap = nc.dram_tensor("wq", [HPC, 128, KP, 2, 128], BF16,
                           kind="ExternalInput").ap()
    wk_ap = nc.dram_tensor("wk", [HPC, 128, KP, 2, 128], BF16,
                           kind="ExternalInput").ap()
    # wv: [p, m, i, ch, j] = wv[128*(2m+i)+p, 512ch+j]
    wv_ap = nc.dram_tensor("wv", [128, KP, 2, 2, 512], BF16,
                           kind="ExternalInput").ap()
    # wp: [p, ci, cc, j] = wp_perm[128ci+p, 512cc+j]
    wp_ap = nc.dram_tensor("wp", [128, 2 * HPC, 2, 512], BF16,
                           kind="ExternalInput").ap()
    maskT_ap = nc.dram_tensor("maskT", [128, 128], BF16,
                              kind="ExternalInput").ap()
    out_ap = nc.dram_tensor("out", [T, CPC], BF16, kind="ExternalOutput").ap()

    with tile.TileContext(nc) as tc:
        _body(nc, tc, x_ap, wq_ap, wk_ap, wv_ap, wp_ap, maskT_ap, out_ap)
    nc.compile()
    return nc


def _qk_head(nc, h, xbs, wts_cur, pools):
    """Emit the q/k gemms for head h."""
    qkp, pa = pools["qkp"], pools["pa"]
    qkT = []
    for wi, nm in ((0, "q"), (1, "k")):
        outT = qkp.tile([128, T], BF16, tag="qkT", name=f"{nm}T{h}")
        qkT.append(outT)
        wt = wts_cur[wi]
        for th in range(2):
            ps = pa.tile([128, 512], F32, tag="pqk")
            sl = slice(512 * th, 512 * (th + 1))
            for k in range(KC):
                nc.tensor.matmul(
                    ps, wt[:, k // 2, k % 2], xbs[k // 2][:, k % 2, sl],
                    start=(k == 0), stop=(k == KC - 1))
            nc.scalar.copy(out=outT[:, sl], in_=ps)
    return qkT


def _s_head(nc, h, qkT, maskT, pools):
    """Scores + exp + causal mask + row-sum pair tiles for head h."""
    Exp = mybir.ActivationFunctionType.Exp
    mult = mybir.AluOpType.mult
    add = mybir.AluOpType.add
    ptp, pss, smp = pools["ptp"], pools["pss"], pools["smp"]
    qTh, kTh = qkT

    pts = []
    for j in range(8):
        w_j = T - 128 * j
        pt = ptp[j].tile([128, w_j], BF16, tag=f"pT{j}", name=f"pT{h}_{j}")
        pts.append(pt)
        off = 128 * j
        while off < T:
            cw = min(512, T - off)
            sp = pss.tile([128, 512], F32, tag="sT")
            nc.tensor.matmul(
                sp[:, :cw], kTh[:, 128 * j:128 * (j + 1)],
                qTh[:, off:off + cw], start=True, stop=True)
            nc.scalar.activation(
                out=pt[:, off - 128 * j:off - 128 * j + cw],
                in_=sp[:, :cw], func=Exp, scale=SCALE)
            off += cw
        # causal mask on the diagonal block (local cols 0:128)
        nc.vector.tensor_tensor(
            out=pt[:, 0:128], in0=pt[:, 0:128], in1=maskT, op=mult)
    # DVE pair-sums for the row-sum gemm: sm[m] = pT(2m) + pT(2m+1),
    # halving the ones-gemm moving stream in _apply_head.
    sms = []
    for m in range(4):
        w_m = T - 256 * m
        sm = smp.tile([128, w_m], BF16, tag=f"sm{m}", name=f"sm{h}_{m}")
        sms.append(sm)
        nc.vector.tensor_copy(out=sm[:, 0:128], in_=pts[2 * m][:, 0:128])
        nc.vector.tensor_tensor(
            out=sm[:, 128:], in0=pts[2 * m][:, 128:],
            in1=pts[2 * m + 1], op=add)
    return pts, sms


def _apply_head(nc, h, pts, sms, vv, ones_col, pools, ytk, yt_loc, yt_all):
    """Attention-apply + normalize + AllGather for head h (lagged)."""
    mult = mybir.AluOpType.mult
    from concourse.dve_ops import RECIPROCAL_APPROX_NR
    psy, psr, asm = pools["psy"], pools["psr"], pools["asm"]

    yt = ytk[h]
    # One psr bank holds both row-sum halves at disjoint partition rows
    # (0 and 32), so g1's accumulation never waits on g0's drain.
    rpt = psr.tile([33, 512], F32, tag="rp")
    for g in range(2):
        tq0 = 512 * g
        jmax = 4 * (g + 1)
        yp = psy.tile([128, 512], F32, tag="yp")
        rp = rpt[32 * g:32 * g + 1]
        for j in range(jmax):
            lo = max(tq0, 128 * j)          # first valid tq
            w = tq0 + 512 - lo
            rhs = pts[j][:, lo - 128 * j:lo - 128 * j + w]
            vblk = vv[j][:, 128 * h:128 * (h + 1)]
            nc.tensor.matmul(
                yp[:, lo - tq0:lo - tq0 + w], vblk, rhs,
                start=(j == 0), stop=(j == jmax - 1))
        for mp in range(jmax // 2):
            lo = max(tq0, 256 * mp)
            w = tq0 + 512 - lo
            rhs = sms[mp][:, lo - 256 * mp:lo - 256 * mp + w]
            nc.tensor.matmul(
                rp[:, lo - tq0:lo - tq0 + w], ones_col, rhs,
                start=(mp == 0), stop=(mp == jmax // 2 - 1))
        # softmax denom: reciprocal on [1,512] first, then broadcast.
        # The PSUM drain copy runs on DVE: on ACT it can sit behind the
        # next head's 13 exp instructions, and g1's row-sum gemm waits on
        # it (PSUM hazards are tracked at tile granularity, not partition
        # ranges); the DVE backlog (masks + pair-sums) is much shorter.
        r_sb = asm.tile([1, 512], F32, tag="r_sb")
        nc.vector.tensor_copy(out=r_sb, in_=rp)
        rec1 = asm.tile([1, 512], F32, tag="rec1")
        nc.vector.reciprocal_approx_fast(out=rec1, in_=r_sb)
        nc.vector._custom_dve(
            RECIPROCAL_APPROX_NR, out=rec1, in0=r_sb, in1=rec1, s0=2.0)
        rec = asm.tile([128, 512], F32, tag="rec")
        nc.gpsimd.partition_broadcast(rec, rec1)
        nc.vector.tensor_tensor(out=yt[:, tq0:tq0 + 512],
                                in0=yp, in1=rec, op=mult)
    # ship this head's yT to the pair as soon as it's done
    nc.sync.dma_start(out=yt_loc[h], in_=yt)
    nc.gpsimd.collective_compute(
        "AllGather", mybir.AluOpType.bypass,
        replica_groups=PAIRS,
        ins=[yt_loc[h].opt()], outs=[yt_all[h].opt()])


def _body(nc, tc, x_ap, wq_ap, wk_ap, wv_ap, wp_ap, maskT_ap, out_ap):
    import contextlib

    with tc.tile_pool(name="const", bufs=1) as const, \
         tc.tile_pool(name="dram", bufs=HPC, space="DRAM") as dram:
        scratch = const.tile([128, 512], BF16, tag="scratch")
        nc.vector.memset(scratch, 0.0)
        ones_f32 = const.tile([128, 1], F32, tag="ones_f32")
        nc.vector.memset(ones_f32, 1.0)
        ones_col = const.tile([128, 1], BF16, tag="ones_col")
        nc.scalar.copy(out=ones_col, in_=ones_f32)

        yt_loc = [dram.tile([128, T], BF16, tag="ytl", name=f"ytl{h}")
                  for h in range(HPC)]
        yt_all = [dram.tile([TP, 128, T], BF16, tag="yta", name=f"yta{h}")
                  for h in range(HPC)]

        # PE warm-up: the first input bytes take ~14us to arrive (DMA
        # spin-up) and the HAM clock gate needs ~3.4us of continuous PE
        # activity to reach 8/8. Run dummy matmuls on the memset scratch
        # so real work starts at full clock.
        with tc.tile_pool(name="warm", bufs=1, space="PSUM") as wps:
            wp_t = wps.tile([1, 512], F32, tag="warm")
            for _ in range(N_WARM):
                nc.tensor.matmul(wp_t, scratch[:, 0:1], scratch,
                                 start=True, stop=True)

        with tc.tile_pool(name="xb", bufs=KP) as xbp, \
             tc.tile_pool(name="wpb", bufs=1) as wpbp, \
             tc.tile_pool(name="vvp", bufs=8) as vvp, \
             tc.tile_pool(name="ytk", bufs=HPC) as ytk_pool, \
             tc.tile_pool(name="wqk", bufs=3) as wqk:
            # x and wv stream as 512KB k-pair tiles interleaved across the
            # sync and gpsimd queues in v-gemm consumption order; wp and
            # maskT ride behind them (needed much later).
            vv = [vvp.tile([128, CPC], BF16, tag="vv", name=f"vv{j}")
                  for j in range(8)]
            ytk = [ytk_pool.tile([128, T], BF16, tag="ytk", name=f"ytk{h}")
                   for h in range(HPC)]

            def load_wqk(h2):
                tiles = []
                for (w_ap, nm) in ((wq_ap, "q"), (wk_ap, "k")):
                    wt = wqk.tile([128, KP, 2, 128], BF16, tag="wqk",
                                  name=f"w{nm}{h2}")
                    nc.sync.dma_start(out=wt, in_=w_ap[h2])
                    tiles.append(wt)
                return tiles

            # ---- v natural [t, c]: stationary x slices, moving wv ----
            # The wv pool closes right after the v stage so its 32KB per
            # partition is recycled for the head-loop pools.
            with tc.tile_pool(name="wvb", bufs=KP) as wvbp, \
                 tc.tile_pool(name="pv", bufs=8, space="PSUM") as pv:
                xbs, wvbs = [], []
                for m in range(KP):
                    xe = nc.sync if m % 2 == 0 else nc.gpsimd
                    we = nc.gpsimd if m % 2 == 0 else nc.sync
                    xt = xbp.tile([128, 2, T], BF16, tag="xb",
                                  name=f"xb{m}")
                    xe.dma_start(out=xt, in_=x_ap[:, m])
                    xbs.append(xt)
                    wt = wvbp.tile([128, 2, 2, 512], BF16, tag="wvb",
                                   name=f"wvb{m}")
                    we.dma_start(out=wt, in_=wv_ap[:, m])
                    wvbs.append(wt)
                maskT = const.tile([128, 128], BF16, tag="maskT")
                nc.sync.dma_start(out=maskT, in_=maskT_ap)
                wpb = wpbp.tile([128, 2 * HPC, 2, 512], BF16, tag="wpb")
                nc.gpsimd.dma_start(out=wpb, in_=wp_ap)
                wts_next = load_wqk(0)

                for ch in range(2):
                    ps = [pv.tile([128, 512], F32, tag="pv",
                                  name=f"pv{ch}_{i}") for i in range(8)]
                    for k in range(KC):
                        xsl = xbs[k // 2][:, k % 2]
                        for tch in range(8):
                            nc.tensor.matmul(
                                ps[tch],
                                xsl[:, 128 * tch:128 * (tch + 1)],
                                wvbs[k // 2][:, k % 2, ch],
                                start=(k == 0), stop=(k == KC - 1))
                            # drain each bank as its accumulation stops,
                            # alternating ACT/DVE so the drains finish in
                            # half the serial time (the head-loop PSUM
                            # pools WAR on the whole pv pool release)
                            if k == KC - 1:
                                dst = vv[tch][:, 512 * ch:512 * (ch + 1)]
                                if tch % 2 == 0:
                                    nc.scalar.copy(out=dst, in_=ps[tch])
                                else:
                                    nc.vector.tensor_copy(out=dst,
                                                          in_=ps[tch])

            # ---- per-head loop ----
            # Emission order per head: qk(h), s(h), apply(h-1). The exp
            # stream of s(h) completes on ACT while the PE runs apply(h-1)
            # and qk(h+1); pT/sm pools are double-buffered so s(h) can
            # write while apply(h-1) still reads the previous head.
            with tc.tile_pool(name="qkp", bufs=3) as qkp, \
                 tc.tile_pool(name="att_sm", bufs=2) as asm, \
                 tc.tile_pool(name="smp", bufs=2) as smp, \
                 tc.tile_pool(name="pa", bufs=2, space="PSUM") as pa, \
                 tc.tile_pool(name="ps_s", bufs=3, space="PSUM") as pss, \
                 tc.tile_pool(name="ps_y", bufs=2, space="PSUM") as psy, \
                 tc.tile_pool(name="ps_r", bufs=1, space="PSUM") as psr, \
                 contextlib.ExitStack() as ptstack:
                ptp = [ptstack.enter_context(
                    tc.tile_pool(name=f"pt{j}", bufs=2))
                    for j in range(8)]
                pools = {"qkp": qkp, "ptp": ptp, "pa": pa,
                         "pss": pss, "psy": psy, "psr": psr,
                         "asm": asm, "smp": smp}

                prev = None
                for h in range(HPC):
                    wts_cur = wts_next
                    if h + 1 < HPC:
                        wts_next = load_wqk(h + 1)
                    qkT = _qk_head(nc, h, xbs, wts_cur, pools)
                    cur = _s_head(nc, h, qkT, maskT, pools)
                    if prev is not None:
                        _apply_head(nc, h - 1, prev[0], prev[1], vv,
                                    ones_col, pools, ytk, yt_loc, yt_all)
                    prev = cur
                _apply_head(nc, HPC - 1, prev[0], prev[1], vv, ones_col,
                            pools, ytk, yt_loc, yt_all)

            # ---- output projection over 16 head-chunks ----
            # Local heads read straight from SBUF (ytk); the peer's 8
            # tiles stream in h-major (AllGather completion order), so
            # only the final head's chunks can ever wait. wp rows are
            # host-permuted: blocks 0-7 = own heads, 8-15 = peer's. The
            # first accumulation group touches the PSUM banks that the
            # head loop frees last (psy/psr) at the end of its m-loop.
            # Output is stored bf16 (host widens); PSUM drains split
            # scalar/vector, stores split sync/gpsimd.
            with tc.tile_pool(name="peer", bufs=HPC) as yfp, \
                 tc.tile_pool(name="out_sb", bufs=4) as osb, \
                 tc.tile_pool(name="ps_o", bufs=8, space="PSUM") as pso:
                prow = 1 - (nc.sync.partition_id() % 2)
                peer = []
                for h2 in range(HPC):
                    t2 = yfp.tile([128, T], BF16, tag="peer",
                                  name=f"peer{h2}")
                    nc.sync.dma_start(
                        out=t2, in_=yt_all[h2][bass.ds(prow, 1)])
                    peer.append(t2)
                for cc in range(2):  # 512-wide output col halves
                    ps = [pso.tile([128, 512], F32, tag="po",
                                   name=f"po{cc}_{m}")
                          for m in range(8)]
                    for ci in range(2 * HPC):
                        ysrc = (ytk[ci] if ci < HPC else peer[ci - HPC])
                        wt = wpb[:, ci, cc]
                        morder = ([0, 1, 2, 3, 4, 7, 5, 6]
                                  if (cc == 0 and ci == 0) else range(8))
                        for m in morder:
                            nc.tensor.matmul(
                                ps[m], ysrc[:, 128 * m:128 * (m + 1)],
                                wt, start=(ci == 0), stop=(ci == 15))
                    for m in range(8):
                        ot = osb.tile([128, 512], BF16, tag="ot")
                        if m % 2 == 0:
                            nc.scalar.copy(out=ot, in_=ps[m])
                        else:
                            nc.vector.tensor_copy(out=ot, in_=ps[m])
                        eng = nc.sync if m % 2 == 0 else nc.gpsimd
                        eng.dma_start(
                            out=out_ap[128 * m:128 * (m + 1),
                                       512 * cc:512 * (cc + 1)],
                            in_=ot)


_NC_CACHE = None


def _get_nc():
    global _NC_CACHE
    if _NC_CACHE is None:
        _NC_CACHE = build_kernel()
    return _NC_CACHE


def kernel(x, w_qkv, w_proj, _trace=False, _trace_kwargs=None):
    x = np.asarray(x, dtype=np.float32)
    w_qkv = np.asarray(w_qkv, dtype=np.float32)
    w_proj = np.asarray(w_proj, dtype=np.float32)

    import ml_dtypes
    bf16 = ml_dtypes.bfloat16
    maskT = np.triu(np.ones((128, 128), dtype=np.float32)).astype(bf16)

    def pack_x(xT):     # [D, T] -> [128, KP, 2, T]
        return np.ascontiguousarray(
            xT.reshape(KP, 2, 128, T).transpose(2, 0, 1, 3)).astype(bf16)

    def pack_w(w):      # [D, CPC] -> [HPC, 128, KP, 2, 128] per head
        return np.ascontiguousarray(
            w.reshape(KP, 2, 128, HPC, 128)
            .transpose(3, 2, 0, 1, 4)).astype(bf16)

    def pack_wv(w):     # [D, CPC] -> [128, KP, 2, 2, 512]
        return np.ascontiguousarray(
            w.reshape(KP, 2, 128, 2, 512).transpose(2, 0, 1, 3, 4)
        ).astype(bf16)

    def pack_wp(w):     # [D, CPC] -> [128, D//128, 2, 512]
        return np.ascontiguousarray(
            w.reshape(D // 128, 128, 2, 512).transpose(1, 0, 2, 3)
        ).astype(bf16)

    in_maps = []
    for c in range(N_CORES):
        b, hh = c // TP, c % TP
        cols = slice(hh * CPC, (hh + 1) * CPC)
        # wp rows permuted so device row-blocks 0-7 are this core's own
        # heads and 8-15 the peer's (kernel chunk order is local-first)
        wp_cols = w_proj[:, cols]
        wp_perm = np.vstack((wp_cols[hh * CPC:(hh + 1) * CPC],
                             wp_cols[(1 - hh) * CPC:(2 - hh) * CPC]))
        in_maps.append({
            "xT": pack_x(np.ascontiguousarray(x[b].T)),
            "wq": pack_w(w_qkv[:, :D][:, cols]),
            "wk": pack_w(w_qkv[:, D:2 * D][:, cols]),
            "wv": pack_wv(w_qkv[:, 2 * D:][:, cols]),
            "wp": pack_wp(wp_perm),
            "maskT": maskT,
        })

    nc = _get_nc()
    res = run_bass_kernel_spmd(nc, in_maps, list(range(N_CORES)),
                               trace=_trace, **(_trace_kwargs or {}))

    out = np.empty((B, T, D), dtype=np.float32)
    for c in range(N_CORES):
        b, hh = c // TP, c % TP
        out[b, :, hh * CPC:(hh + 1) * CPC] = \
            res.results[c]["out"].astype(np.float32)
    if _trace:
        return out, res
    return out


# revision 36
# speedup vs baseline: 1.0051x; 1.0051x over previous
"""Causal self-attention (B=4, T=1024, D=2048, H=16) on 8 trn2 NeuronCores.

Sharding: data-parallel over batch (4) x tensor-parallel over heads (2).
Core c handles batch b = c//2, head-half hh = c%2 (heads hh*8 .. hh*8+8).

All gemms bf16 (fp8 fails the 2e-2 gate: logit/value quantization noise
concentrates in peaked-softmax rows). Perf comes from keeping TensorE
dense at the warm 2.4 GHz clock:
  - ~72 warm-up matmuls on a memset scratch run during the ~14us DMA
    spin-up so the HAM clock gate reaches 8/8 before real work starts.
  - x and wv stream as 512KB tiles interleaved across the sync and
    gpsimd DMA queues in v-gemm consumption order; scalar/vector queues
    issue no DMA, so PSUM-drain copies are never stuck behind descriptor
    waits.
  - per head the emission order is qk(h), scores(h), apply(h-1): the ACT
    exp stream of head h hides under apply(h-1) + qk(h+1). pT pools are
    double-buffered to allow the overlap.
  - softmax row sums use DVE pair-summed pT tiles, halving the ones-gemm
    stream (2560 vs 4608 cycles/head).

Per-core plan (fp32 PSUM accumulation everywhere):
  v      [t, c]  : lhsT = x k-tile slice [k,t], rhs = wv [k,c]
  per head h:
    qT/kT [d, t] : lhsT = w_{q,k} k-tile, rhs = x k-tile [128,512]
    sT    [tk,tq]: lhsT = kT block, rhs = qT slice (causal: tq >= 128*j)
    pT    = exp(scale * sT) via ACT (no max-subtraction; |s*scale| <= 7)
    diag blocks masked multiplicatively with an upper-triangular 0/1 mask
    sm    [tk,tq]: DVE pair-sums pT(2m)+pT(2m+1) for the row-sum gemm
    yT    [d, tq] += v_j-gemm: lhsT = v block, rhs = pT block (PSUM accum)
    r     [1, tq] += ones^T @ sm (softmax row sums, halved stream)
    1/r on [1,512] (DVE recip + NR) -> GpSimd bcast -> yT_norm = yT * rec
    pairwise AllGather of this head's yT (overlaps later heads' compute)
  out    [t, c_half]: accumulate 16 head-chunks against wp rows; the yT
  tiles are prefetched from the AllGather outputs in completion order
  (h-major), so only head 7's chunks can ever wait on the collective.

Host side: slice/transpose/cast/pack inputs per core, concat outputs.
"""

import numpy as np

import concourse.bass as bass
import concourse.mybir as mybir
import concourse.tile as tile
from concourse import bacc
from concourse.bass_utils import run_bass_kernel_spmd

B, T, D = 4, 1024, 2048
H, DH = 16, 128
N_CORES = 8
TP = 2                      # head-halves per batch
HPC = H // TP               # heads per core = 8
CPC = HPC * DH              # channels per core = 1024
KC = D // 128               # contraction chunks = 16
KP = KC // 2                # contraction pair chunks = 8
SCALE = 1.0 / float(np.sqrt(DH))
N_WARM = 24                 # PE warm-up matmuls during DMA spin-up

F32 = mybir.dt.float32
BF16 = mybir.dt.bfloat16

PAIRS = [[2 * i, 2 * i + 1] for i in range(B)]


def build_kernel():
    nc = bacc.Bacc("TRN2", target_bir_lowering=False, debug=False,
                   num_devices=N_CORES)

    # All inputs host-packed to device layout.
    # x: [p, m, i, t] = x.T[128*(2m+i)+p, t]
    x_ap = nc.dram_tensor("xT", [128, KP, 2, T], BF16,
                          kind="ExternalInput").ap()
    # wq/wk per head: [h, p, m, i, j] = w[128*(2m+i)+p, 128h+j]
    wq_ap = nc.dram_tensor("wq", [HPC, 128, KP, 2, 128], BF16,
                           kind="ExternalInput").ap()
    wk_ap = nc.dram_tensor("wk", [HPC, 128, KP, 2, 128], BF16,
                           kind="ExternalInput").ap()
    # wv: [p, m, i, ch, j] = wv[128*(2m+i)+p, 512ch+j]
    wv_ap = nc.dram_tensor("wv", [128, KP, 2, 2, 512], BF16,
                           kind="ExternalInput").ap()
    # wp: [p, ci, cc, j] = wp_perm[128ci+p, 512cc+j]
    wp_ap = nc.dram_tensor("wp", [128, 2 * HPC, 2, 512], BF16,
                           kind="ExternalInput").ap()
    maskT_ap = nc.dram_tensor("maskT", [128, 128], BF16,
                              kind="ExternalInput").ap()
    out_ap = nc.dram_tensor("out", [T, CPC], BF16, kind="ExternalOutput").ap()

    with tile.TileContext(nc) as tc:
        _body(nc, tc, x_ap, wq_ap, wk_ap, wv_ap, wp_ap, maskT_ap, out_ap)
    nc.compile()
    return nc


def _qk_head(nc, h, xbs, wts_cur, pools):
    """Emit the q/k gemms for head h."""
    qkp, pa = pools["qkp"], pools["pa"]
    qkT = []
    for wi, nm in ((0, "q"), (1, "k")):
        outT = qkp.tile([128, T], BF16, tag="qkT", name=f"{nm}T{h}")
        qkT.append(outT)
        wt = wts_cur[wi]
        for th in range(2):
            ps = pa.tile([128, 512], F32, tag="pqk")
            sl = slice(512 * th, 512 * (th + 1))
            for k in range(KC):
                nc.tensor.matmul(
                    ps, wt[:, k // 2, k % 2], xbs[k // 2][:, k % 2, sl],
                    start=(k == 0), stop=(k == KC - 1))
            nc.scalar.copy(out=outT[:, sl], in_=ps)
    return qkT


def _s_head(nc, h, qkT, maskT, pools):
    """Scores + exp + causal mask + row-sum pair tiles for head h."""
    Exp = mybir.ActivationFunctionType.Exp
    mult = mybir.AluOpType.mult
    add = mybir.AluOpType.add
    ptp, pss, smp = pools["ptp"], pools["pss"], pools["smp"]
    qTh, kTh = qkT

    pts = []
    for j in range(8):
        w_j = T - 128 * j
        pt = ptp[j].tile([128, w_j], BF16, tag=f"pT{j}", name=f"pT{h}_{j}")
        pts.append(pt)
        off = 128 * j
        while off < T:
            cw = min(512, T - off)
            sp = pss.tile([128, 512], F32, tag="sT")
            nc.tensor.matmul(
                sp[:, :cw], kTh[:, 128 * j:128 * (j + 1)],
                qTh[:, off:off + cw], start=True, stop=True)
            nc.scalar.activation(
                out=pt[:, off - 128 * j:off - 128 * j + cw],
                in_=sp[:, :cw], func=Exp, scale=SCALE)
            off += cw
        # causal mask on the diagonal block (local cols 0:128)
        nc.vector.tensor_tensor(
            out=pt[:, 0:128], in0=pt[:, 0:128], in1=maskT, op=mult)
    # DVE pair-sums for the row-sum gemm: sm[m] = pT(2m) + pT(2m+1),
    # halving the ones-gemm moving stream in _apply_head.
    sms = []
    for m in range(4):
        w_m = T - 256 * m
        sm = smp.tile([128, w_m], BF16, tag=f"sm{m}", name=f"sm{h}_{m}")
        sms.append(sm)
        nc.vector.tensor_copy(out=sm[:, 0:128], in_=pts[2 * m][:, 0:128])
        nc.vector.tensor_tensor(
            out=sm[:, 128:], in0=pts[2 * m][:, 128:],
            in1=pts[2 * m + 1], op=add)
    return pts, sms


def _apply_head(nc, h, pts, sms, vv, ones_col, pools, ytk, yt_loc, yt_all,
                tail=False):
    """Attention-apply + normalize + AllGather for head h (lagged)."""
    mult = mybir.AluOpType.mult
    from concourse.dve_ops import RECIPROCAL_APPROX_NR
    psy, psr, asm = pools["psy"], pools["psr"], pools["asm"]

    yt = ytk[h]
    # One psr bank holds both row-sum halves at disjoint partition rows
    # (0 and 32), so g1's accumulation never waits on g0's drain.
    rpt = psr.tile([33, 512], F32, tag="rp")
    for g in range(2):
        tq0 = 512 * g
        jmax = 4 * (g + 1)
        yp = psy.tile([128, 512], F32, tag="yp")
        rp = rpt[32 * g:32 * g + 1]
        for j in range(jmax):
            lo = max(tq0, 128 * j)          # first valid tq
            w = tq0 + 512 - lo
            rhs = pts[j][:, lo - 128 * j:lo - 128 * j + w]
            vblk = vv[j][:, 128 * h:128 * (h + 1)]
            nc.tensor.matmul(
                yp[:, lo - tq0:lo - tq0 + w], vblk, rhs,
                start=(j == 0), stop=(j == jmax - 1))
        for mp in range(jmax // 2):
            lo = max(tq0, 256 * mp)
            w = tq0 + 512 - lo
            rhs = sms[mp][:, lo - 256 * mp:lo - 256 * mp + w]
            nc.tensor.matmul(
                rp[:, lo - tq0:lo - tq0 + w], ones_col, rhs,
                start=(mp == 0), stop=(mp == jmax // 2 - 1))
        # softmax denom: reciprocal on [1,512] first, then broadcast.
        # The PSUM drain copy runs on ACT so freeing the psr bank never
        # waits behind queued DVE work (masks/pair-sums of the next
        # head) — except for the tail heads, where ACT is backlogged
        # with the final head's 13 exps and DVE is the shorter queue.
        r_sb = asm.tile([1, 512], F32, tag="r_sb")
        if tail:
            nc.vector.tensor_copy(out=r_sb, in_=rp)
        else:
            nc.scalar.copy(out=r_sb, in_=rp)
        rec1 = asm.tile([1, 512], F32, tag="rec1")
        nc.vector.reciprocal_approx_fast(out=rec1, in_=r_sb)
        nc.vector._custom_dve(
            RECIPROCAL_APPROX_NR, out=rec1, in0=r_sb, in1=rec1, s0=2.0)
        rec = asm.tile([128, 512], F32, tag="rec")
        nc.gpsimd.partition_broadcast(rec, rec1)
        nc.vector.tensor_tensor(out=yt[:, tq0:tq0 + 512],
                                in0=yp, in1=rec, op=mult)
    # ship this head's yT to the pair as soon as it's done
    nc.sync.dma_start(out=yt_loc[h], in_=yt)
    nc.gpsimd.collective_compute(
        "AllGather", mybir.AluOpType.bypass,
        replica_groups=PAIRS,
        ins=[yt_loc[h].opt()], outs=[yt_all[h].opt()])


def _body(nc, tc, x_ap, wq_ap, wk_ap, wv_ap, wp_ap, maskT_ap, out_ap):
    import contextlib

    with tc.tile_pool(name="const", bufs=1) as const, \
         tc.tile_pool(name="dram", bufs=HPC, space="DRAM") as dram:
        scratch = const.tile([128, 512], BF16, tag="scratch")
        nc.vector.memset(scratch, 0.0)
        ones_f32 = const.tile([128, 1], F32, tag="ones_f32")
        nc.vector.memset(ones_f32, 1.0)
        ones_col = const.tile([128, 1], BF16, tag="ones_col")
        nc.scalar.copy(out=ones_col, in_=ones_f32)

        yt_loc = [dram.tile([128, T], BF16, tag="ytl", name=f"ytl{h}")
                  for h in range(HPC)]
        yt_all = [dram.tile([TP, 128, T], BF16, tag="yta", name=f"yta{h}")
                  for h in range(HPC)]

        # PE warm-up: the first input bytes take ~14us to arrive (DMA
        # spin-up) and the HAM clock gate needs ~3.4us of continuous PE
        # activity to reach 8/8. Run dummy matmuls on the memset scratch
        # so real work starts at full clock.
        with tc.tile_pool(name="warm", bufs=1, space="PSUM") as wps:
            wp_t = wps.tile([1, 512], F32, tag="warm")
            for _ in range(N_WARM):
                nc.tensor.matmul(wp_t, scratch[:, 0:1], scratch,
                                 start=True, stop=True)

        with tc.tile_pool(name="xb", bufs=KP) as xbp, \
             tc.tile_pool(name="wpb", bufs=1) as wpbp, \
             tc.tile_pool(name="vvp", bufs=8) as vvp, \
             tc.tile_pool(name="ytk", bufs=HPC) as ytk_pool, \
             tc.tile_pool(name="wqk", bufs=3) as wqk:
            # x and wv stream as 512KB k-pair tiles interleaved across the
            # sync and gpsimd queues in v-gemm consumption order; wp and
            # maskT ride behind them (needed much later).
            vv = [vvp.tile([128, CPC], BF16, tag="vv", name=f"vv{j}")
                  for j in range(8)]
            ytk = [ytk_pool.tile([128, T], BF16, tag="ytk", name=f"ytk{h}")
                   for h in range(HPC)]

            def load_wqk(h2):
                tiles = []
                for (w_ap, nm) in ((wq_ap, "q"), (wk_ap, "k")):
                    wt = wqk.tile([128, KP, 2, 128], BF16, tag="wqk",
                                  name=f"w{nm}{h2}")
                    nc.sync.dma_start(out=wt, in_=w_ap[h2])
                    tiles.append(wt)
                return tiles

            # ---- v natural [t, c]: stationary x slices, moving wv ----
            # The wv pool closes right after the v stage so its 32KB per
            # partition is recycled for the head-loop pools.
            with tc.tile_pool(name="wvb", bufs=KP) as wvbp, \
                 tc.tile_pool(name="pv", bufs=8, space="PSUM") as pv:
                xbs, wvbs = [], []
                for m in range(KP):
                    xe = nc.sync if m % 2 == 0 else nc.gpsimd
                    we = nc.gpsimd if m % 2 == 0 else nc.sync
                    xt = xbp.tile([128, 2, T], BF16, tag="xb",
                                  name=f"xb{m}")
                    xe.dma_start(out=xt, in_=x_ap[:, m])
                    xbs.append(xt)
                    wt = wvbp.tile([128, 2, 2, 512], BF16, tag="wvb",
                                   name=f"wvb{m}")
                    we.dma_start(out=wt, in_=wv_ap[:, m])
                    wvbs.append(wt)
                maskT = const.tile([128, 128], BF16, tag="maskT")
                nc.sync.dma_start(out=maskT, in_=maskT_ap)
                wpb = wpbp.tile([128, 2 * HPC, 2, 512], BF16, tag="wpb")
                nc.gpsimd.dma_start(out=wpb, in_=wp_ap)
                wts_next = load_wqk(0)

                for ch in range(2):
                    ps = [pv.tile([128, 512], F32, tag="pv",
                                  name=f"pv{ch}_{i}") for i in range(8)]
                    for k in range(KC):
                        xsl = xbs[k // 2][:, k % 2]
                        for tch in range(8):
                            nc.tensor.matmul(
                                ps[tch],
                                xsl[:, 128 * tch:128 * (tch + 1)],
                                wvbs[k // 2][:, k % 2, ch],
                                start=(k == 0), stop=(k == KC - 1))
                            # drain each bank as its accumulation stops,
                            # ch1 alternating ACT/DVE: the head-loop PSUM
                            # pools WAR on the whole pv pool release, so
                            # halving the serial drain time pulls qk(0)
                            # forward
                            if k == KC - 1:
                                dst = vv[tch][:, 512 * ch:512 * (ch + 1)]
                                if ch == 1 and tch % 2 == 1:
                                    nc.vector.tensor_copy(out=dst,
                                                          in_=ps[tch])
                                else:
                                    nc.scalar.copy(out=dst, in_=ps[tch])

            # ---- per-head loop ----
            # Emission order per head: qk(h), s(h), apply(h-1). The exp
            # stream of s(h) completes on ACT while the PE runs apply(h-1)
            # and qk(h+1); pT/sm pools are double-buffered so s(h) can
            # write while apply(h-1) still reads the previous head.
            with tc.tile_pool(name="qkp", bufs=3) as qkp, \
                 tc.tile_pool(name="att_sm", bufs=2) as asm, \
                 tc.tile_pool(name="smp", bufs=2) as smp, \
                 tc.tile_pool(name="pa", bufs=2, space="PSUM") as pa, \
                 tc.tile_pool(name="ps_s", bufs=3, space="PSUM") as pss, \
                 tc.tile_pool(name="ps_y", bufs=2, space="PSUM") as psy, \
                 tc.tile_pool(name="ps_r", bufs=1, space="PSUM") as psr, \
                 contextlib.ExitStack() as ptstack:
                ptp = [ptstack.enter_context(
                    tc.tile_pool(name=f"pt{j}", bufs=2))
                    for j in range(8)]
                pools = {"qkp": qkp, "ptp": ptp, "pa": pa,
                         "pss": pss, "psy": psy, "psr": psr,
                         "asm": asm, "smp": smp}

                prev = None
                for h in range(HPC):
                    wts_cur = wts_next
                    if h + 1 < HPC:
                        wts_next = load_wqk(h + 1)
                    qkT = _qk_head(nc, h, xbs, wts_cur, pools)
                    cur = _s_head(nc, h, qkT, maskT, pools)
                    if prev is not None:
                        _apply_head(nc, h - 1, prev[0], prev[1], vv,
                                    ones_col, pools, ytk, yt_loc, yt_all,
                                    tail=(h == HPC - 1))
                    prev = cur
                _apply_head(nc, HPC - 1, prev[0], prev[1], vv, ones_col,
                            pools, ytk, yt_loc, yt_all, tail=True)

            # ---- output projection over 16 head-chunks ----
            # Local heads read straight from SBUF (ytk); the peer's 8
            # tiles stream in h-major (AllGather completion order), so
            # only the final head's chunks can ever wait. wp rows are
            # host-permuted: blocks 0-7 = own heads, 8-15 = peer's. The
            # first accumulation group touches the PSUM banks that the
            # head loop frees last (psy/psr) at the end of its m-loop.
            # Output is stored bf16 (host widens); PSUM drains split
            # scalar/vector, stores split sync/gpsimd.
            with tc.tile_pool(name="peer", bufs=HPC) as yfp, \
                 tc.tile_pool(name="out_sb", bufs=4) as osb, \
                 tc.tile_pool(name="ps_o", bufs=8, space="PSUM") as pso:
                prow = 1 - (nc.sync.partition_id() % 2)
                peer = []
                for h2 in range(HPC):
                    t2 = yfp.tile([128, T], BF16, tag="peer",
                                  name=f"peer{h2}")
                    nc.sync.dma_start(
                        out=t2, in_=yt_all[h2][bass.ds(prow, 1)])
                    peer.append(t2)
                for cc in range(2):  # 512-wide output col halves
                    ps = [pso.tile([128, 512], F32, tag="po",
                                   name=f"po{cc}_{m}")
                          for m in range(8)]
                    for ci in range(2 * HPC):
                        ysrc = (ytk[ci] if ci < HPC else peer[ci - HPC])
                        wt = wpb[:, ci, cc]
                        morder = ([0, 1, 2, 3, 4, 7, 5, 6]
                                  if (cc == 0 and ci == 0) else range(8))
                        for m in morder:
                            nc.tensor.matmul(
                                ps[m], ysrc[:, 128 * m:128 * (m + 1)],
                                wt, start=(ci == 0), stop=(ci == 15))
                    for m in range(8):
                        ot = osb.tile([128, 512], BF16, tag="ot")
                        if m % 2 == 0:
                            nc.scalar.copy(out=ot, in_=ps[m])
                        else:
                            nc.vector.tensor_copy(out=ot, in_=ps[m])
                        eng = nc.sync if m % 2 == 0 else nc.gpsimd
                        eng.dma_start(
                            out=out_ap[128 * m:128 * (m + 1),
                                       512 * cc:512 * (cc + 1)],
                            in_=ot)


_NC_CACHE = None


def _get_nc():
    global _NC_CACHE
    if _NC_CACHE is None:
        _NC_CACHE = build_kernel()
    return _NC_CACHE


def kernel(x, w_qkv, w_proj, _trace=False, _trace_kwargs=None):
    x = np.asarray(x, dtype=np.float32)
    w_qkv = np.asarray(w_qkv, dtype=np.float32)
    w_proj = np.asarray(w_proj, dtype=np.float32)

    import ml_dtypes
    bf16 = ml_dtypes.bfloat16
    maskT = np.triu(np.ones((128, 128), dtype=np.float32)).astype(bf16)

    def pack_x(xT):     # [D, T] -> [128, KP, 2, T]
        return np.ascontiguousarray(
            xT.reshape(KP, 2, 128, T).transpose(2, 0, 1, 3)).astype(bf16)

    def pack_w(w):      # [D, CPC] -> [HPC, 128, KP, 2, 128] per head
        return np.ascontiguousarray(
            w.reshape(KP, 2, 128, HPC, 128)
            .transpose(3, 2, 0, 1, 4)).astype(bf16)

    def pack_wv(w):     # [D, CPC] -> [128, KP, 2, 2, 512]
        return np.ascontiguousarray(
            w.reshape(KP, 2, 128, 2, 512).transpose(2, 0, 1, 3, 4)
        ).astype(bf16)

    def pack_wp(w):     # [D, CPC] -> [128, D//128, 2, 512]
        return np.ascontiguousarray(
            w.reshape(D // 128, 128, 2, 512).transpose(1, 0, 2, 3)
        ).astype(bf16)

    in_maps = []
    for c in range(N_CORES):
        b, hh = c // TP, c % TP
        cols = slice(hh * CPC, (hh + 1) * CPC)
        # wp rows permuted so device row-blocks 0-7 are this core's own
        # heads and 8-15 the peer's (kernel chunk order is local-first)
        wp_cols = w_proj[:, cols]
        wp_perm = np.vstack((wp_cols[hh * CPC:(hh + 1) * CPC],
                             wp_cols[(1 - hh) * CPC:(2 - hh) * CPC]))
        in_maps.append({
            "xT": pack_x(np.ascontiguousarray(x[b].T)),
            "wq": pack_w(w_qkv[:, :D][:, cols]),
            "wk": pack_w(w_qkv[:, D:2 * D][:, cols]),
            "wv": pack_wv(w_qkv[:, 2 * D:][:, cols]),
            "wp": pack_wp(wp_perm),
            "maskT": maskT,
        })

    nc = _get_nc()
    res = run_bass_kernel_spmd(nc, in_maps, list(range(N_CORES)),
                               trace=_trace, **(_trace_kwargs or {}))

    out = np.empty((B, T, D), dtype=np.float32)
    for c in range(N_CORES):
        b, hh = c // TP, c % TP
        out[b, :, hh * CPC:(hh + 1) * CPC] = \
            res.results[c]["out"].astype(np.float32)
    if _trace:
        return out, res
    return out


# revision 37
# speedup vs baseline: 1.0166x; 1.0114x over previous
"""Causal self-attention (B=4, T=1024, D=2048, H=16) on 8 trn2 NeuronCores.

Sharding: data-parallel over batch (4) x tensor-parallel over heads (2).
Core c handles batch b = c//2, head-half hh = c%2 (heads hh*8 .. hh*8+8).

All gemms bf16 (fp8 fails the 2e-2 gate: logit/value quantization noise
concentrates in peaked-softmax rows). Perf comes from keeping TensorE
dense at the warm 2.4 GHz clock:
  - ~72 warm-up matmuls on a memset scratch run during the ~14us DMA
    spin-up so the HAM clock gate reaches 8/8 before real work starts.
  - x and wv stream as 512KB tiles interleaved across the sync and
    gpsimd DMA queues in v-gemm consumption order; scalar/vector queues
    issue no DMA, so PSUM-drain copies are never stuck behind descriptor
    waits.
  - per head the emission order is qk(h), scores(h), apply(h-1): the ACT
    exp stream of head h hides under apply(h-1) + qk(h+1). pT pools are
    double-buffered to allow the overlap.
  - softmax row sums use DVE pair-summed pT tiles, halving the ones-gemm
    stream (2560 vs 4608 cycles/head).

Per-core plan (fp32 PSUM accumulation everywhere):
  v      [t, c]  : lhsT = x k-tile slice [k,t], rhs = wv [k,c]
  per head h:
    qT/kT [d, t] : lhsT = w_{q,k} k-tile, rhs = x k-tile [128,512]
    sT    [tk,tq]: lhsT = kT block, rhs = qT slice (causal: tq >= 128*j)
    pT    = exp(scale * sT) via ACT (no max-subtraction; |s*scale| <= 7)
    diag blocks masked multiplicatively with an upper-triangular 0/1 mask
    sm    [tk,tq]: DVE pair-sums pT(2m)+pT(2m+1) for the row-sum gemm
    yT    [d, tq] += v_j-gemm: lhsT = v block, rhs = pT block (PSUM accum)
    r     [1, tq] += ones^T @ sm (softmax row sums, halved stream)
    1/r on [1,512] (DVE recip + NR) -> GpSimd bcast -> yT_norm = yT * rec
    pairwise AllGather of this head's yT (overlaps later heads' compute)
  out    [t, c_half]: accumulate 16 head-chunks against wp rows; the yT
  tiles are prefetched from the AllGather outputs in completion order
  (h-major), so only head 7's chunks can ever wait on the collective.

Host side: slice/transpose/cast/pack inputs per core, concat outputs.
"""

import numpy as np

import concourse.bass as bass
import concourse.mybir as mybir
import concourse.tile as tile
from concourse import bacc
from concourse.bass_utils import run_bass_kernel_spmd

B, T, D = 4, 1024, 2048
H, DH = 16, 128
N_CORES = 8
TP = 2                      # head-halves per batch
HPC = H // TP               # heads per core = 8
CPC = HPC * DH              # channels per core = 1024
KC = D // 128               # contraction chunks = 16
KP = KC // 2                # contraction pair chunks = 8
SCALE = 1.0 / float(np.sqrt(DH))
N_WARM = 24                 # PE warm-up matmuls during DMA spin-up

F32 = mybir.dt.float32
BF16 = mybir.dt.bfloat16

PAIRS = [[2 * i, 2 * i + 1] for i in range(B)]


def build_kernel():
    nc = bacc.Bacc("TRN2", target_bir_lowering=False, debug=False,
                   num_devices=N_CORES)

    # All inputs host-packed to device layout.
    # x: [p, m, i, t] = x.T[128*(2m+i)+p, t]
    x_ap = nc.dram_tensor("xT", [128, KP, 2, T], BF16,
                          kind="ExternalInput").ap()
    # wq/wk per head: [h, p, m, i, j] = w[128*(2m+i)+p, 128h+j]
    wq_ap = nc.dram_tensor("wq", [HPC, 128, KP, 2, 128], BF16,
                           kind="ExternalInput").ap()
    wk_ap = nc.dram_tensor("wk", [HPC, 128, KP, 2, 128], BF16,
                           kind="ExternalInput").ap()
    # wv: [p, m, i, ch, j] = wv[128*(2m+i)+p, 512ch+j]
    wv_ap = nc.dram_tensor("wv", [128, KP, 2, 2, 512], BF16,
                           kind="ExternalInput").ap()
    # wp: [p, ci, cc, j] = wp_perm[128ci+p, 512cc+j]
    wp_ap = nc.dram_tensor("wp", [128, 2 * HPC, 2, 512], BF16,
                           kind="ExternalInput").ap()
    maskT_ap = nc.dram_tensor("maskT", [128, 128], BF16,
                              kind="ExternalInput").ap()
    out_ap = nc.dram_tensor("out", [T, CPC], BF16, kind="ExternalOutput").ap()

    with tile.TileContext(nc) as tc:
        _body(nc, tc, x_ap, wq_ap, wk_ap, wv_ap, wp_ap, maskT_ap, out_ap)
    nc.compile()
    return nc


def _qk_head(nc, h, xbs, wts_cur, pools):
    """Emit the q/k gemms for head h."""
    qkp, pa = pools["qkp"], pools["pa"]
    qkT = []
    for wi, nm in ((0, "q"), (1, "k")):
        outT = qkp.tile([128, T], BF16, tag="qkT", name=f"{nm}T{h}")
        qkT.append(outT)
        wt = wts_cur[wi]
        for th in range(2):
            ps = pa.tile([128, 512], F32, tag="pqk")
            sl = slice(512 * th, 512 * (th + 1))
            for k in range(KC):
                nc.tensor.matmul(
                    ps, wt[:, k // 2, k % 2], xbs[k // 2][:, k % 2, sl],
                    start=(k == 0), stop=(k == KC - 1))
            nc.scalar.copy(out=outT[:, sl], in_=ps)
    return qkT


def _s_head(nc, h, qkT, maskT, pools):
    """Scores + exp + causal mask + row-sum pair tiles for head h."""
    Exp = mybir.ActivationFunctionType.Exp
    mult = mybir.AluOpType.mult
    add = mybir.AluOpType.add
    ptp, pss, smp = pools["ptp"], pools["pss"], pools["smp"]
    qTh, kTh = qkT

    pts = []
    for j in range(8):
        w_j = T - 128 * j
        pt = ptp[j].tile([128, w_j], BF16, tag=f"pT{j}", name=f"pT{h}_{j}")
        pts.append(pt)
        off = 128 * j
        while off < T:
            cw = min(512, T - off)
            sp = pss.tile([128, 512], F32, tag="sT")
            nc.tensor.matmul(
                sp[:, :cw], kTh[:, 128 * j:128 * (j + 1)],
                qTh[:, off:off + cw], start=True, stop=True)
            nc.scalar.activation(
                out=pt[:, off - 128 * j:off - 128 * j + cw],
                in_=sp[:, :cw], func=Exp, scale=SCALE)
            off += cw
        # causal mask on the diagonal block (local cols 0:128)
        nc.vector.tensor_tensor(
            out=pt[:, 0:128], in0=pt[:, 0:128], in1=maskT, op=mult)
    # DVE pair-sums for the row-sum gemm: sm[m] = pT(2m) + pT(2m+1),
    # halving the ones-gemm moving stream in _apply_head.
    sms = []
    for m in range(4):
        w_m = T - 256 * m
        sm = smp.tile([128, w_m], BF16, tag=f"sm{m}", name=f"sm{h}_{m}")
        sms.append(sm)
        nc.vector.tensor_copy(out=sm[:, 0:128], in_=pts[2 * m][:, 0:128])
        nc.vector.tensor_tensor(
            out=sm[:, 128:], in0=pts[2 * m][:, 128:],
            in1=pts[2 * m + 1], op=add)
    return pts, sms


def _apply_head(nc, h, pts, sms, vv, ones_col, pools, ytk, yt_loc, yt_all,
                tail=False):
    """Attention-apply + normalize + AllGather for head h (lagged)."""
    mult = mybir.AluOpType.mult
    from concourse.dve_ops import RECIPROCAL_APPROX_NR
    psy, psr, asm = pools["psy"], pools["psr"], pools["asm"]

    yt = ytk[h]
    # One psr bank holds both row-sum halves at disjoint partition rows
    # (0 and 32), so g1's accumulation never waits on g0's drain.
    rpt = psr.tile([33, 512], F32, tag="rp")
    for g in range(2):
        tq0 = 512 * g
        jmax = 4 * (g + 1)
        yp = psy.tile([128, 512], F32, tag="yp")
        rp = rpt[32 * g:32 * g + 1]
        for j in range(jmax):
            lo = max(tq0, 128 * j)          # first valid tq
            w = tq0 + 512 - lo
            rhs = pts[j][:, lo - 128 * j:lo - 128 * j + w]
            vblk = vv[j][:, 128 * h:128 * (h + 1)]
            nc.tensor.matmul(
                yp[:, lo - tq0:lo - tq0 + w], vblk, rhs,
                start=(j == 0), stop=(j == jmax - 1))
        for mp in range(jmax // 2):
            lo = max(tq0, 256 * mp)
            w = tq0 + 512 - lo
            rhs = sms[mp][:, lo - 256 * mp:lo - 256 * mp + w]
            nc.tensor.matmul(
                rp[:, lo - tq0:lo - tq0 + w], ones_col, rhs,
                start=(mp == 0), stop=(mp == jmax // 2 - 1))
        # softmax denom: reciprocal on [1,512] first, then broadcast.
        # The PSUM drain copy runs on ACT so freeing the psr bank never
        # waits behind queued DVE work (masks/pair-sums of the next
        # head) — except for the tail heads, where ACT is backlogged
        # with the final head's 13 exps and DVE is the shorter queue.
        r_sb = asm.tile([1, 512], F32, tag="r_sb")
        nc.scalar.copy(out=r_sb, in_=rp)
        rec1 = asm.tile([1, 512], F32, tag="rec1")
        nc.vector.reciprocal_approx_fast(out=rec1, in_=r_sb)
        nc.vector._custom_dve(
            RECIPROCAL_APPROX_NR, out=rec1, in0=r_sb, in1=rec1, s0=2.0)
        rec = asm.tile([128, 512], F32, tag="rec")
        nc.gpsimd.partition_broadcast(rec, rec1)
        nc.vector.tensor_tensor(out=yt[:, tq0:tq0 + 512],
                                in0=yp, in1=rec, op=mult)
    # ship this head's yT to the pair as soon as it's done
    nc.sync.dma_start(out=yt_loc[h], in_=yt)
    nc.gpsimd.collective_compute(
        "AllGather", mybir.AluOpType.bypass,
        replica_groups=PAIRS,
        ins=[yt_loc[h].opt()], outs=[yt_all[h].opt()])


def _body(nc, tc, x_ap, wq_ap, wk_ap, wv_ap, wp_ap, maskT_ap, out_ap):
    import contextlib

    with tc.tile_pool(name="const", bufs=1) as const, \
         tc.tile_pool(name="dram", bufs=HPC, space="DRAM") as dram:
        scratch = const.tile([128, 512], BF16, tag="scratch")
        nc.vector.memset(scratch, 0.0)
        ones_f32 = const.tile([128, 1], F32, tag="ones_f32")
        nc.vector.memset(ones_f32, 1.0)
        ones_col = const.tile([128, 1], BF16, tag="ones_col")
        nc.scalar.copy(out=ones_col, in_=ones_f32)

        yt_loc = [dram.tile([128, T], BF16, tag="ytl", name=f"ytl{h}")
                  for h in range(HPC)]
        yt_all = [dram.tile([TP, 128, T], BF16, tag="yta", name=f"yta{h}")
                  for h in range(HPC)]

        # PE warm-up: the first input bytes take ~14us to arrive (DMA
        # spin-up) and the HAM clock gate needs ~3.4us of continuous PE
        # activity to reach 8/8. Run dummy matmuls on the memset scratch
        # so real work starts at full clock.
        with tc.tile_pool(name="warm", bufs=1, space="PSUM") as wps:
            wp_t = wps.tile([1, 512], F32, tag="warm")
            for _ in range(N_WARM):
                nc.tensor.matmul(wp_t, scratch[:, 0:1], scratch,
                                 start=True, stop=True)

        with tc.tile_pool(name="xb", bufs=KP) as xbp, \
             tc.tile_pool(name="wpb", bufs=1) as wpbp, \
             tc.tile_pool(name="vvp", bufs=8) as vvp, \
             tc.tile_pool(name="ytk", bufs=HPC) as ytk_pool, \
             tc.tile_pool(name="wqk", bufs=3) as wqk:
            # x and wv stream as 512KB k-pair tiles interleaved across the
            # sync and gpsimd queues in v-gemm consumption order; wp and
            # maskT ride behind them (needed much later).
            vv = [vvp.tile([128, CPC], BF16, tag="vv", name=f"vv{j}")
                  for j in range(8)]
            ytk = [ytk_pool.tile([128, T], BF16, tag="ytk", name=f"ytk{h}")
                   for h in range(HPC)]

            def load_wqk(h2):
                tiles = []
                for (w_ap, nm) in ((wq_ap, "q"), (wk_ap, "k")):
                    wt = wqk.tile([128, KP, 2, 128], BF16, tag="wqk",
                                  name=f"w{nm}{h2}")
                    nc.sync.dma_start(out=wt, in_=w_ap[h2])
                    tiles.append(wt)
                return tiles

            # ---- v natural [t, c]: stationary x slices, moving wv ----
            # The wv pool closes right after the v stage so its 32KB per
            # partition is recycled for the head-loop pools.
            with tc.tile_pool(name="wvb", bufs=KP) as wvbp, \
                 tc.tile_pool(name="pv", bufs=8, space="PSUM") as pv:
                xbs, wvbs = [], []
                for m in range(KP):
                    xe = nc.sync if m % 2 == 0 else nc.gpsimd
                    we = nc.gpsimd if m % 2 == 0 else nc.sync
                    xt = xbp.tile([128, 2, T], BF16, tag="xb",
                                  name=f"xb{m}")
                    xe.dma_start(out=xt, in_=x_ap[:, m])
                    xbs.append(xt)
                    wt = wvbp.tile([128, 2, 2, 512], BF16, tag="wvb",
                                   name=f"wvb{m}")
                    we.dma_start(out=wt, in_=wv_ap[:, m])
                    wvbs.append(wt)
                maskT = const.tile([128, 128], BF16, tag="maskT")
                nc.sync.dma_start(out=maskT, in_=maskT_ap)
                wpb = wpbp.tile([128, 2 * HPC, 2, 512], BF16, tag="wpb")
                nc.gpsimd.dma_start(out=wpb, in_=wp_ap)
                wts_next = load_wqk(0)

                for ch in range(2):
                    ps = [pv.tile([128, 512], F32, tag="pv",
                                  name=f"pv{ch}_{i}") for i in range(8)]
                    for k in range(KC):
                        xsl = xbs[k // 2][:, k % 2]
                        for tch in range(8):
                            nc.tensor.matmul(
                                ps[tch],
                                xsl[:, 128 * tch:128 * (tch + 1)],
                                wvbs[k // 2][:, k % 2, ch],
                                start=(k == 0), stop=(k == KC - 1))
                            # drain each bank as its accumulation stops,
                            # ch1 alternating ACT/DVE: the head-loop PSUM
                            # pools WAR on the whole pv pool release, so
                            # halving the serial drain time pulls qk(0)
                            # forward
                            if k == KC - 1:
                                dst = vv[tch][:, 512 * ch:512 * (ch + 1)]
                                if ch == 1 and tch % 2 == 1:
                                    nc.vector.tensor_copy(out=dst,
                                                          in_=ps[tch])
                                else:
                                    nc.scalar.copy(out=dst, in_=ps[tch])

            # ---- per-head loop ----
            # Emission order per head: qk(h), s(h), apply(h-1). The exp
            # stream of s(h) completes on ACT while the PE runs apply(h-1)
            # and qk(h+1); pT/sm pools are double-buffered so s(h) can
            # write while apply(h-1) still reads the previous head.
            with tc.tile_pool(name="qkp", bufs=3) as qkp, \
                 tc.tile_pool(name="att_sm", bufs=2) as asm, \
                 tc.tile_pool(name="smp", bufs=2) as smp, \
                 tc.tile_pool(name="pa", bufs=2, space="PSUM") as pa, \
                 tc.tile_pool(name="ps_s", bufs=3, space="PSUM") as pss, \
                 tc.tile_pool(name="ps_y", bufs=2, space="PSUM") as psy, \
                 tc.tile_pool(name="ps_r", bufs=1, space="PSUM") as psr, \
                 contextlib.ExitStack() as ptstack:
                ptp = [ptstack.enter_context(
                    tc.tile_pool(name=f"pt{j}", bufs=2))
                    for j in range(8)]
                pools = {"qkp": qkp, "ptp": ptp, "pa": pa,
                         "pss": pss, "psy": psy, "psr": psr,
                         "asm": asm, "smp": smp}

                prev = None
                for h in range(HPC):
                    wts_cur = wts_next
                    if h + 1 < HPC:
                        wts_next = load_wqk(h + 1)
                    qkT = _qk_head(nc, h, xbs, wts_cur, pools)
                    cur = _s_head(nc, h, qkT, maskT, pools)
                    if prev is not None:
                        _apply_head(nc, h - 1, prev[0], prev[1], vv,
                                    ones_col, pools, ytk, yt_loc, yt_all,
                                    tail=(h == HPC - 1))
                    prev = cur
                _apply_head(nc, HPC - 1, prev[0], prev[1], vv, ones_col,
                            pools, ytk, yt_loc, yt_all, tail=True)

            # ---- output projection over 16 head-chunks ----
            # Local heads read straight from SBUF (ytk); the peer's 8
            # tiles stream in h-major (AllGather completion order), so
            # only the final head's chunks can ever wait. wp rows are
            # host-permuted: blocks 0-7 = own heads, 8-15 = peer's. The
            # first accumulation group touches the PSUM banks that the
            # head loop frees last (psy/psr) at the end of its m-loop.
            # Output is stored bf16 (host widens); PSUM drains split
            # scalar/vector, stores split sync/gpsimd.
            with tc.tile_pool(name="peer", bufs=HPC) as yfp, \
                 tc.tile_pool(name="out_sb", bufs=4) as osb, \
                 tc.tile_pool(name="ps_o", bufs=8, space="PSUM") as pso:
                prow = 1 - (nc.sync.partition_id() % 2)
                peer = []
                for h2 in range(HPC):
                    t2 = yfp.tile([128, T], BF16, tag="peer",
                                  name=f"peer{h2}")
                    nc.sync.dma_start(
                        out=t2, in_=yt_all[h2][bass.ds(prow, 1)])
                    peer.append(t2)
                for cc in range(2):  # 512-wide output col halves
                    ps = [pso.tile([128, 512], F32, tag="po",
                                   name=f"po{cc}_{m}")
                          for m in range(8)]
                    for ci in range(2 * HPC):
                        ysrc = (ytk[ci] if ci < HPC else peer[ci - HPC])
                        wt = wpb[:, ci, cc]
                        morder = ([0, 1, 2, 3, 4, 7, 5, 6]
                                  if (cc == 0 and ci == 0) else range(8))
                        for m in morder:
                            nc.tensor.matmul(
                                ps[m], ysrc[:, 128 * m:128 * (m + 1)],
                                wt, start=(ci == 0), stop=(ci == 15))
                    for m in range(8):
                        ot = osb.tile([128, 512], BF16, tag="ot")
                        if m % 2 == 0:
                            nc.scalar.copy(out=ot, in_=ps[m])
                        else:
                            nc.vector.tensor_copy(out=ot, in_=ps[m])
                        eng = nc.sync if m % 2 == 0 else nc.gpsimd
                        eng.dma_start(
                            out=out_ap[128 * m:128 * (m + 1),
                                       512 * cc:512 * (cc + 1)],
                            in_=ot)


_NC_CACHE = None


def _get_nc():
    global _NC_CACHE
    if _NC_CACHE is None:
        _NC_CACHE = build_kernel()
    return _NC_CACHE


def kernel(x, w_qkv, w_proj, _trace=False, _trace_kwargs=None):
    x = np.asarray(x, dtype=np.float32)
    w_qkv = np.asarray(w_qkv, dtype=np.float32)
    w_proj = np.asarray(w_proj, dtype=np.float32)

    import ml_dtypes
    bf16 = ml_dtypes.bfloat16
    maskT = np.triu(np.ones((128, 128), dtype=np.float32)).astype(bf16)

    def pack_x(xT):     # [D, T] -> [128, KP, 2, T]
        return np.ascontiguousarray(
            xT.reshape(KP, 2, 128, T).transpose(2, 0, 1, 3)).astype(bf16)

    def pack_w(w):      # [D, CPC] -> [HPC, 128, KP, 2, 128] per head
        return np.ascontiguousarray(
            w.reshape(KP, 2, 128, HPC, 128)
            .transpose(3, 2, 0, 1, 4)).astype(bf16)

    def pack_wv(w):     # [D, CPC] -> [128, KP, 2, 2, 512]
        return np.ascontiguousarray(
            w.reshape(KP, 2, 128, 2, 512).transpose(2, 0, 1, 3, 4)
        ).astype(bf16)

    def pack_wp(w):     # [D, CPC] -> [128, D//128, 2, 512]
        return np.ascontiguousarray(
            w.reshape(D // 128, 128, 2, 512).transpose(1, 0, 2, 3)
        ).astype(bf16)

    in_maps = []
    for c in range(N_CORES):
        b, hh = c // TP, c % TP
        cols = slice(hh * CPC, (hh + 1) * CPC)
        # wp rows permuted so device row-blocks 0-7 are this core's own
        # heads and 8-15 the peer's (kernel chunk order is local-first)
        wp_cols = w_proj[:, cols]
        wp_perm = np.vstack((wp_cols[hh * CPC:(hh + 1) * CPC],
                             wp_cols[(1 - hh) * CPC:(2 - hh) * CPC]))
        in_maps.append({
            "xT": pack_x(np.ascontiguousarray(x[b].T)),
            "wq": pack_w(w_qkv[:, :D][:, cols]),
            "wk": pack_w(w_qkv[:, D:2 * D][:, cols]),
            "wv": pack_wv(w_qkv[:, 2 * D:][:, cols]),
            "wp": pack_wp(wp_perm),
            "maskT": maskT,
        })

    nc = _get_nc()
    res = run_bass_kernel_spmd(nc, in_maps, list(range(N_CORES)),
                               trace=_trace, **(_trace_kwargs or {}))

    out = np.empty((B, T, D), dtype=np.float32)
    for c in range(N_CORES):
        b, hh = c // TP, c % TP
        out[b, :, hh * CPC:(hh + 1) * CPC] = \
            res.results[c]["out"].astype(np.float32)
    if _trace:
        return out, res
    return out
